# revision 7
# baseline (speedup 1.0000x reference)
"""GAT (4-layer, 8-head) Trainium2 kernel, 8-core SPMD — v2.

Strategy: nodes partitioned into 8 contiguous shards (4096/core); each core
owns the edges whose dst falls in its shard (segment softmax + scatter stay
local). Per layer, each core computes ft = h @ W for its shard in bf16, packs
[ft(512) | el(8) | er(8) | pad] into 640-bf16 rows, and one AllGather
replicates them (Shared output) so every core can dma_gather remote src rows.
Edge aggregation = PE matmuls against one-hot dst matrices (bf16), with
exp(leaky(el+er)) folded into the moving operand.

v2 over v1: bf16 exchange/gather/scatter path (halves DMA + collective
bytes), h kept resident in SBUF across layers (no HBM round-trips), dense(l+1)
chunks interleaved into edge(l) blocks so PE/DVE/DMA overlap, all weights +
gather indices + graph one-hots preloaded to SBUF, bf16 layer-3 exchange via
elem_step-packed gathers.

Softmax: reference subtracts per-segment max; alpha is shift-invariant and
logits are small, so exp() directly in f32 and normalize by segment sum.
"""

import functools

import numpy as np

import concourse.bacc as bacc
import concourse.bass as bass
import concourse.mybir as mybir
import concourse.tile as tile
from concourse.bass_utils import run_bass_kernel_spmd

# ---- problem constants (hardcoded per contract) ----
N, E, G = 32768, 262144, 64
NCORES = 8
SH = N // NCORES          # 4096 nodes per core
NB = SH // 128            # 32 dst blocks per core
F0, F = 128, 512
H, D = 8, 64
H3 = 6
FR = F + 128              # packed bf16 row: ft(512) | el(8) | er(8) | pad
NEG_SLOPE = 0.2
EPS = 1e-30

f32 = mybir.dt.float32
f32r = mybir.dt.float32r
bf16 = mybir.dt.bfloat16
i16 = mybir.dt.int16

TRACE = False
TRACE_KW = {}
LAST = {}

AF = mybir.ActivationFunctionType
ALU = mybir.AluOpType
AX = mybir.AxisListType


def _wrap_idx(v):
    """int16 gather-index layout: element i at [i%16, i//16], replicated to
    128 partitions."""
    L = len(v)
    w = np.zeros((16, L // 16), np.int16)
    w[np.arange(L) % 16, np.arange(L) // 16] = v.astype(np.int16)
    return np.tile(w, (8, 1))


def _bf16():
    try:
        return np.dtype("bfloat16")
    except TypeError:
        import ml_dtypes
        return ml_dtypes.bfloat16


def preprocess(inputs):
    src = np.asarray(inputs["src"]).astype(np.int64)
    dst = np.asarray(inputs["dst"]).astype(np.int64)
    graph_id = np.asarray(inputs["graph_id"]).astype(np.int64)
    feat = np.asarray(inputs["feat"], dtype=np.float32)
    bfnp = _bf16()

    per_core_edges = []
    KB = 0
    for c in range(NCORES):
        m = (dst >= c * SH) & (dst < (c + 1) * SH)
        es, ed = src[m], dst[m]
        o = np.argsort(ed, kind="stable")
        es, ed = es[o], ed[o]
        dl = ed - c * SH
        blk = dl >> 7
        counts = np.bincount(blk, minlength=NB)
        KB = max(KB, int(np.ceil(counts.max() / 128)))
        per_core_edges.append((es, ed, dl, blk, counts))
    EB = KB * 128

    # shared weight-derived arrays
    def Amat(al):  # [1,H,D] -> [H*D, H]
        al = np.asarray(al, np.float64)[0]
        hh, dd = al.shape
        A = np.zeros((hh * dd, hh), np.float64)
        for h in range(hh):
            A[h * dd:(h + 1) * dd, h] = al[h]
        return A

    W0 = np.asarray(inputs["W0"], np.float64)
    W1 = np.asarray(inputs["W1"], np.float64)
    W2 = np.asarray(inputs["W2"], np.float64)
    W3 = np.asarray(inputs["W3"], np.float64)
    resW3 = np.asarray(inputs["resW3"], np.float64)
    al3 = np.asarray(inputs["al3"], np.float64)[0, :, 0]
    ar3 = np.asarray(inputs["ar3"], np.float64)[0, :, 0]

    WA = {}
    for l, W in ((0, W0), (1, W1), (2, W2)):
        Aal = Amat(inputs[f"al{l}"])
        Aar = Amat(inputs[f"ar{l}"])
        WA[l] = np.concatenate([W @ Aal, W @ Aar], axis=1).astype(bfnp)
    W3c = np.concatenate(
        [W3, W3 * al3[None, :], W3 * ar3[None, :], resW3], axis=1
    ).astype(bfnp)  # [512, 24]
    b3row = np.zeros((1, 24), np.float32)
    b3row[0, 18:24] = np.asarray(inputs["b3"], np.float32)

    bias_bc = np.tile(
        np.concatenate([np.asarray(inputs[f"b{l}"], np.float32)
                        for l in range(3)])[None, :], (128, 1))  # [128, 3*512]
    lin_bc = np.zeros((128, H3 + 1), np.float32)
    lin_bc[:, 0:H3] = np.asarray(inputs["linW"], np.float32)[:, 0][None, :]
    lin_bc[:, H3] = float(np.asarray(inputs["linb"], np.float32)[0])

    shared = {
        "W0": W0.astype(bfnp),
        "W1": W1.astype(bfnp),
        "W2": W2.astype(bfnp),
        "W3c": W3c,
        "WA0": WA[0], "WA1": WA[1], "WA2": WA[2],
        "b3row": b3row.astype(bfnp),
        "bias_bc": bias_bc,
        "lin_bc": lin_bc,
        "identity": np.eye(128, dtype=bfnp),
        "ones1": np.ones((1, 128), bfnp),
    }

    in_maps = []
    eye64 = np.eye(G, dtype=np.float32)
    for c in range(NCORES):
        es, ed, dl, blk, counts = per_core_edges[c]
        offs = np.concatenate([[0], np.cumsum(counts)])
        idxX = np.zeros((128, NB * EB // 16), np.int16)
        idxE = np.zeros((128, NB * EB // 16), np.int16)
        Sarr = np.zeros((NB, 128, EB), np.float32)
        for b in range(NB):
            s_b = es[offs[b]:offs[b + 1]]
            d_b = ed[offs[b]:offs[b + 1]]
            dloc = dl[offs[b]:offs[b + 1]] - b * 128
            npad = EB - len(s_b)
            s_pad = np.concatenate([s_b, np.zeros(npad, np.int64)])
            dg_pad = np.concatenate([d_b, np.zeros(npad, np.int64)])
            idxX[:, b * EB // 16:(b + 1) * EB // 16] = _wrap_idx(s_pad)
            idxE[:, b * EB // 16:(b + 1) * EB // 16] = _wrap_idx(dg_pad)
            j = np.arange(len(dloc))
            S3 = Sarr[b].reshape(128, KB, 128)
            S3[j % 128, j // 128, dloc] = 1.0
        gid = graph_id[c * SH:(c + 1) * SH]
        Gh = eye64[gid].reshape(NB, 128, G)
        im = dict(shared)
        im["feat_sh"] = feat[c * SH:(c + 1) * SH].astype(bfnp)
        im["idxX"] = idxX
        im["idxE"] = idxE
        im["Sarr"] = Sarr.astype(bfnp)
        im["Gh"] = Gh
        in_maps.append(im)
    return in_maps, KB


@functools.lru_cache(maxsize=8)
def build_program(KB, phases=5, no_cc=False, ebufs=3, t2a_dve=True):
    EB = KB * 128
    nc = bacc.Bacc("TRN2", target_bir_lowering=False, debug=False)

    # ---- I/O ----
    feat_sh = nc.dram_tensor("feat_sh", [SH, F0], bf16, kind="ExternalInput")
    Wt = {
        0: nc.dram_tensor("W0", [F0, F], bf16, kind="ExternalInput"),
        1: nc.dram_tensor("W1", [F, F], bf16, kind="ExternalInput"),
        2: nc.dram_tensor("W2", [F, F], bf16, kind="ExternalInput"),
        3: nc.dram_tensor("W3c", [F, 24], bf16, kind="ExternalInput"),
    }
    WAt = {l: nc.dram_tensor(f"WA{l}", [F0 if l == 0 else F, 16], bf16,
                             kind="ExternalInput") for l in range(3)}
    b3row = nc.dram_tensor("b3row", [1, 24], bf16, kind="ExternalInput")
    bias_bc = nc.dram_tensor("bias_bc", [128, 3 * F], f32, kind="ExternalInput")
    lin_bc = nc.dram_tensor("lin_bc", [128, H3 + 1], f32, kind="ExternalInput")
    identity = nc.dram_tensor("identity", [128, 128], bf16, kind="ExternalInput")
    ones1 = nc.dram_tensor("ones1", [1, 128], bf16, kind="ExternalInput")
    idxX = nc.dram_tensor("idxX", [128, NB * EB // 16], i16, kind="ExternalInput")
    idxE = nc.dram_tensor("idxE", [128, NB * EB // 16], i16, kind="ExternalInput")
    Sarr = nc.dram_tensor("Sarr", [NB, 128, EB], bf16, kind="ExternalInput")
    Gh = nc.dram_tensor("Gh", [NB, 128, G], f32r, kind="ExternalInput")
    out = nc.dram_tensor("out", [G, 1], f32, kind="ExternalOutput")

    rg = [list(range(NCORES))]

    with tile.TileContext(nc) as tc:
        with (
            tc.tile_pool(name="const", bufs=1) as constp,
            tc.tile_pool(name="work", bufs=2) as work,
            tc.tile_pool(name="edge", bufs=ebufs) as edge,
            tc.tile_pool(name="psA", bufs=2, space="PSUM") as psA,
            tc.tile_pool(name="psB", bufs=2, space="PSUM") as psB,
            tc.tile_pool(name="psC", bufs=2, space="PSUM") as psC,
            tc.tile_pool(name="psP", bufs=1, space="PSUM") as psP,
            tc.tile_pool(name="dram", bufs=1, space="DRAM") as dram,
        ):
            # ---- resident constants ----
            ident_sb = constp.tile([128, 128], bf16)
            nc.sync.dma_start(ident_sb[:], identity[:])
            ones_sb = constp.tile([1, 128], bf16)
            nc.sync.dma_start(ones_sb[:], ones1[:])
            b3r_sb = constp.tile([1, 24], bf16)
            nc.sync.dma_start(b3r_sb[:], b3row[:])
            lin_sb = constp.tile([128, H3 + 1], f32)
            nc.sync.dma_start(lin_sb[:], lin_bc[:])
            bias_sb = constp.tile([128, 3 * F], f32)
            nc.sync.dma_start(bias_sb[:], bias_bc[:])
            res3_sb = constp.tile([128, NB * H3], f32)
            ixX_sb = constp.tile([128, NB * EB // 16], i16)
            nc.sync.dma_start(ixX_sb[:], idxX[:])
            ixE_sb = constp.tile([128, NB * EB // 16], i16)
            nc.sync.dma_start(ixE_sb[:], idxE[:])
            gh_sb = constp.tile([128, NB, G], f32r)
            nc.sync.dma_start(gh_sb[:], Gh[:].rearrange("n p g -> p n g"))
            w_sb = {}
            wa_sb = {}
            for l in range(4):
                K = F0 if l == 0 else F
                KBl = K // 128
                FW = F if l < 3 else 24
                w_sb[l] = constp.tile([128, KBl, FW], bf16, name=f"w{l}sb")
                nc.sync.dma_start(
                    w_sb[l][:], Wt[l][:].rearrange("(kb p) f -> p kb f", p=128))
                if l < 3:
                    wa_sb[l] = constp.tile([128, KBl, 16], bf16, name=f"wa{l}sb")
                    nc.sync.dma_start(
                        wa_sb[l][:],
                        WAt[l][:].rearrange("(kb p) f -> p kb f", p=128))

            # ---- resident node features (ping-pong) ----
            hA = constp.tile([128, NB, F], bf16)   # h1, then h3
            hB = constp.tile([128, NB, F], bf16)   # h2

            # ---- internal DRAM arrays ----
            ftag = {l: dram.tile([SH, FR], bf16, name=f"ftag{l}")
                    for l in range(3)}
            ftg = {l: dram.tile([N, FR], bf16, name=f"ftg{l}",
                                addr_space="Shared") for l in range(3)}
            ft3ag = dram.tile([SH, 64], f32, name="ft3ag")
            ft3g = dram.tile([N, 64], f32, name="ft3g", addr_space="Shared")
            ar_in = dram.tile([G, H3], f32, name="arin")
            ar_out = dram.tile([G, H3], f32, name="arout", addr_space="Shared")

            def hbuf(l):
                """SBUF buffer holding h_{l} (the INPUT of layer l)."""
                return (hA, hB, hA)[l - 1] if l >= 1 else None

            # ---- dense chunk: h(l) chunk j -> ftag[l] rows / ft3ag ----
            def dense_chunk(l, j):
                K = F0 if l == 0 else F
                KBl = K // 128
                if l == 0:
                    hsrc = work.tile([128, F0], bf16, tag="hload")
                    nc.sync.dma_start(hsrc[:], feat_sh[j * 128:(j + 1) * 128, :])
                else:
                    hsrc = hbuf(l)[:, j, :]
                pT = psA.tile([128, K], bf16, tag="A")
                for kb in range(KBl):
                    nc.tensor.transpose(
                        pT[:, kb * 128:(kb + 1) * 128],
                        hsrc[:, kb * 128:(kb + 1) * 128], ident_sb[:])
                hT = work.tile([128, K], bf16, tag="hT")
                nc.scalar.copy(hT[:], pT[:])
                FW = F if l < 3 else 24
                pft = psB.tile([128, FW], f32, tag="B")
                for kb in range(KBl):
                    nc.tensor.matmul(
                        pft[:], hT[:, kb * 128:(kb + 1) * 128],
                        w_sb[l][:, kb, :],
                        start=(kb == 0), stop=(kb == KBl - 1 and l < 3))
                if l == 3:
                    nc.tensor.matmul(pft[:], ones_sb[:], b3r_sb[:],
                                     start=False, stop=True)
                    ft3t = work.tile([128, 64], f32, tag="ft3sb")
                    nc.any.tensor_copy(ft3t[:, 0:18], pft[:, 0:18])
                    nc.any.tensor_copy(
                        res3_sb[:, j * H3:(j + 1) * H3], pft[:, 18:24])
                    nc.sync.dma_start(ft3ag[j * 128:(j + 1) * 128, :], ft3t[:])
                else:
                    pel = psC.tile([128, 16], f32, tag="C")
                    for kb in range(KBl):
                        nc.tensor.matmul(
                            pel[:], hT[:, kb * 128:(kb + 1) * 128],
                            wa_sb[l][:, kb, :],
                            start=(kb == 0), stop=(kb == KBl - 1))
                    ftt = work.tile([128, FR], bf16, tag="ftsb")
                    nc.scalar.copy(ftt[:, 0:F], pft[:])
                    nc.scalar.copy(ftt[:, F:F + 16], pel[:])
                    nc.sync.dma_start(ftag[l][j * 128:(j + 1) * 128, :], ftt[:])

            def allgather(l):
                if no_cc:
                    return
                if l == 3:
                    nc.gpsimd.collective_compute(
                        "AllGather", ALU.bypass, replica_groups=rg,
                        ins=[ft3ag[:].opt()], outs=[ft3g[:].opt()])
                else:
                    nc.gpsimd.collective_compute(
                        "AllGather", ALU.bypass, replica_groups=rg,
                        ins=[ftag[l][:].opt()], outs=[ftg[l][:].opt()])

            # ---- edge block: aggregate layer l for dst block b -> h(l+1) ----
            def edge_block(l, b):
                S_t = edge.tile([128, KB, 128], bf16, tag="S")
                nc.sync.dma_start(
                    S_t[:], Sarr[b].rearrange("p (c d) -> p c d", d=128))
                X = edge.tile([128, KB, FR], bf16, tag="X")
                nc.gpsimd.dma_gather(
                    X[:], ftg[l][:], ixX_sb[:, b * EB // 16:(b + 1) * EB // 16],
                    num_idxs=EB, num_idxs_reg=EB, elem_size=FR,
                    single_packet=False)
                EL = edge.tile([128, KB, 128], bf16, tag="EL")
                nc.gpsimd.dma_gather(
                    EL[:], ftg[l][:, F:FR],
                    ixE_sb[:, b * EB // 16:(b + 1) * EB // 16],
                    num_idxs=EB, num_idxs_reg=EB, elem_size=128,
                    elem_step=FR, single_packet=False)
                et = edge.tile([128, KB, H], f32, tag="et")
                nc.vector.tensor_tensor(
                    et[:], X[:, 0:KB, F:F + H], EL[:, 0:KB, H:2 * H], ALU.add)
                lt = edge.tile([128, KB * H], f32, tag="lt")
                nc.vector.scalar_tensor_tensor(
                    lt[:], et[:].rearrange("p c h -> p (c h)"), NEG_SLOPE,
                    et[:].rearrange("p c h -> p (c h)"), ALU.mult, ALU.max)
                pt = edge.tile([128, KB * H], bf16, tag="pt")
                nc.scalar.activation(pt[:], lt[:], AF.Exp)
                Xv = X[:, :, 0:F].rearrange("p c (h d) -> p c h d", h=H)
                pb = pt[:].rearrange("p (c h) -> p c h", h=H) \
                    .unsqueeze(3).broadcast_to([128, KB, H, D])
                nc.vector.tensor_tensor(Xv, Xv, pb, ALU.mult)
                prst = psA.tile([128, F], f32, tag="A")
                ps = psC.tile([128, H], f32, tag="C")
                for c in range(KB):
                    nc.tensor.matmul(
                        prst[:], S_t[:, c, :], X[:, c, 0:F],
                        start=(c == 0), stop=(c == KB - 1))
                for c in range(KB):
                    nc.tensor.matmul(
                        ps[:], S_t[:, c, :], pt[:, c * H:(c + 1) * H],
                        start=(c == 0), stop=(c == KB - 1))
                sse = edge.tile([128, H], f32, tag="sse")
                nc.vector.tensor_scalar_add(sse[:], ps[:], EPS)
                rs = edge.tile([128, H], f32, tag="rs")
                nc.vector.reciprocal(rs[:], sse[:])
                t1 = edge.tile([128, H, D], f32, tag="t1")
                nc.vector.tensor_tensor(
                    t1[:], prst[:].rearrange("p (h d) -> p h d", h=H),
                    rs[:].unsqueeze(2).broadcast_to([128, H, D]), ALU.mult)
                t1f = t1[:].rearrange("p h d -> p (h d)")
                t2 = edge.tile([128, F], f32, tag="t2")
                if l == 0:
                    nc.vector.tensor_tensor(
                        t2[:], t1f, bias_sb[:, l * F:(l + 1) * F], ALU.add)
                else:
                    t2a = edge.tile([128, F], f32, tag="t2a")
                    eng = nc.vector if t2a_dve else nc.gpsimd
                    eng.tensor_tensor(
                        t2a[:], t1f, hbuf(l)[:, b, :], ALU.add)
                    nc.vector.tensor_tensor(
                        t2[:], t2a[:], bias_sb[:, l * F:(l + 1) * F], ALU.add)
                # ELU -> h(l+1) resident in SBUF
                mm = edge.tile([128, F], f32, tag="mm")
                nc.vector.tensor_scalar_min(mm[:], t2[:], 0.0)
                ex = edge.tile([128, F], f32, tag="ex")
                nc.scalar.activation(ex[:], mm[:], AF.Exp)
                rl = edge.tile([128, F], f32, tag="rl")
                nc.scalar.activation(rl[:], t2[:], AF.Relu)
                nc.vector.scalar_tensor_tensor(
                    hbuf(l + 1)[:, b, :], ex[:], -1.0, rl[:], ALU.add, ALU.add)

            # ---- edge block layer 3 + pooling ----
            ppool = psP.tile([G, H3], f32, tag="P")

            def edge3_block(b):
                S_t = edge.tile([128, KB, 128], bf16, tag="S")
                nc.sync.dma_start(
                    S_t[:], Sarr[b].rearrange("p (c d) -> p c d", d=128))
                XS = edge.tile([128, KB, 64], f32, tag="X3")
                nc.gpsimd.dma_gather(
                    XS[:], ft3g[:], ixX_sb[:, b * EB // 16:(b + 1) * EB // 16],
                    num_idxs=EB, num_idxs_reg=EB, elem_size=64,
                    single_packet=False)
                EL = edge.tile([128, KB, 64], f32, tag="EL3")
                nc.gpsimd.dma_gather(
                    EL[:], ft3g[:], ixE_sb[:, b * EB // 16:(b + 1) * EB // 16],
                    num_idxs=EB, num_idxs_reg=EB, elem_size=64,
                    single_packet=False)
                et = edge.tile([128, KB, H3], f32, tag="et")
                nc.vector.tensor_tensor(
                    et[:], XS[:, 0:KB, 6:12], EL[:, 0:KB, 12:18], ALU.add)
                lt = edge.tile([128, KB * H3], f32, tag="lt")
                nc.vector.scalar_tensor_tensor(
                    lt[:], et[:].rearrange("p c h -> p (c h)"), NEG_SLOPE,
                    et[:].rearrange("p c h -> p (c h)"), ALU.mult, ALU.max)
                XP = edge.tile([128, KB, 2 * H3], bf16, tag="pt3")
                nc.scalar.activation(
                    XP[:, :, H3:2 * H3],
                    lt[:].rearrange("p (c h) -> p c h", h=H3), AF.Exp)
                nc.vector.tensor_tensor(
                    XP[:, :, 0:H3], XS[:, 0:KB, 0:H3], XP[:, :, H3:2 * H3],
                    ALU.mult)
                prst = psC.tile([128, 2 * H3], f32, tag="C")
                for c in range(KB):
                    nc.tensor.matmul(
                        prst[:], S_t[:, c, :], XP[:, c, :],
                        start=(c == 0), stop=(c == KB - 1))
                sse = edge.tile([128, H3], f32, tag="sse")
                nc.vector.tensor_scalar_add(sse[:], prst[:, H3:2 * H3], EPS)
                rs = edge.tile([128, H3], f32, tag="rs")
                nc.vector.reciprocal(rs[:], sse[:])
                t1 = edge.tile([128, H3], f32, tag="t1")
                nc.vector.tensor_tensor(t1[:], prst[:, 0:H3], rs[:], ALU.mult)
                h3 = edge.tile([128, H3], f32r, tag="hn3")
                nc.vector.tensor_tensor(
                    h3[:], t1[:], res3_sb[:, b * H3:(b + 1) * H3], ALU.add)
                nc.tensor.matmul(
                    ppool[:], gh_sb[:, b, :], h3[:],
                    start=(b == 0), stop=(b == NB - 1))

            def readout():
                pol = work.tile([G, H3], f32, tag="pol")
                nc.any.tensor_copy(pol[:], ppool[:])
                nc.sync.dma_start(ar_in[:], pol[:])
                nc.gpsimd.collective_compute(
                    "AllReduce", ALU.add, replica_groups=rg,
                    ins=[ar_in[:].opt()], outs=[ar_out[:].opt()])
                pol2 = work.tile([G, H3], f32, tag="pol2")
                nc.sync.dma_start(pol2[:], ar_out[:])
                pr = work.tile([G, H3], f32, tag="pr")
                nc.vector.tensor_tensor(
                    pr[:], pol2[:], lin_sb[0:G, 0:H3], ALU.mult)
                ro = work.tile([G, 1], f32, tag="ro")
                nc.vector.tensor_reduce(ro[:], pr[:], axis=AX.X, op=ALU.add)
                ro2 = work.tile([G, 1], f32, tag="ro2")
                nc.vector.tensor_tensor(
                    ro2[:], ro[:], lin_sb[0:G, H3:H3 + 1], ALU.add)
                nc.sync.dma_start(out[:], ro2[:])

            # ---- schedule ----
            def stage0():
                for j in range(NB):
                    dense_chunk(0, j)
                allgather(0)

            def stage(l):  # edge(l) blocks interleaved with dense(l+1)
                for b in range(NB):
                    edge_block(l, b)
                    dense_chunk(l + 1, b)
                allgather(l + 1)

            def stage3():
                for b in range(NB):
                    edge3_block(b)
                readout()

            steps = [("dense0", stage0),
                     ("layer0", lambda: stage(0)),
                     ("layer1", lambda: stage(1)),
                     ("layer2", lambda: stage(2)),
                     ("layer3", stage3)]
            for nm, st in steps[:phases]:
                with nc.named_scope(nm):
                    st()

    nc.compile()
    return nc


def kernel(**inputs):
    in_maps, KB = preprocess(inputs)
    nc = build_program(KB, LAST.get("phases", 5), LAST.get("no_cc", False),
                       LAST.get("ebufs", 3), LAST.get("t2a_dve", True))
    br = run_bass_kernel_spmd(
        nc, in_maps, core_ids=list(range(NCORES)), trace=TRACE, **TRACE_KW)
    LAST["br"] = br
    return np.asarray(br.results[0]["out"], dtype=np.float32)


# revision 17
# speedup vs baseline: 2.8128x; 2.8128x over previous
"""GAT (4-layer, 8-head) Trainium2 kernel, 8-core SPMD — v2.

Strategy: nodes partitioned into 8 contiguous shards (4096/core); each core
owns the edges whose dst falls in its shard (segment softmax + scatter stay
local). Per layer, each core computes ft = h @ W for its shard in bf16, packs
[ft(512) | el(8) | er(8) | pad] into 640-bf16 rows, and one AllGather
replicates them (Shared output) so every core can dma_gather remote src rows.
Edge aggregation = PE matmuls against one-hot dst matrices (bf16), with
exp(leaky(el+er)) folded into the moving operand.

v2 over v1: bf16 exchange/gather/scatter path (halves DMA + collective
bytes), h kept resident in SBUF across layers (no HBM round-trips), dense(l+1)
chunks interleaved into edge(l) blocks so PE/DVE/DMA overlap, all weights +
gather indices + graph one-hots preloaded to SBUF, bf16 layer-3 exchange via
elem_step-packed gathers.

Softmax: reference subtracts per-segment max; alpha is shift-invariant and
logits are small, so exp() directly in f32 and normalize by segment sum.
"""

import functools

import numpy as np

import concourse.bacc as bacc
import concourse.bass as bass
import concourse.mybir as mybir
import concourse.tile as tile
from concourse.bass_utils import run_bass_kernel_spmd

# ---- problem constants (hardcoded per contract) ----
N, E, G = 32768, 262144, 64
NCORES = 8
SH = N // NCORES          # 4096 nodes per core
NB = SH // 128            # 32 dst blocks per core
F0, F = 128, 512
H, D = 8, 64
H3 = 6
FR = 768                  # packed fp8 row bytes: ft(512 fp8) | el+er(16 bf16) | pad
NEG_SLOPE = 0.2
EPS = 1e-30

f32 = mybir.dt.float32
f32r = mybir.dt.float32r
bf16 = mybir.dt.bfloat16
fp8 = mybir.dt.float8e4
i16 = mybir.dt.int16

TRACE = False
TRACE_KW = {}
LAST = {}

AF = mybir.ActivationFunctionType
ALU = mybir.AluOpType
AX = mybir.AxisListType


def _wrap_idx(v):
    """int16 gather-index layout: element i at [i%16, i//16], replicated to
    128 partitions."""
    L = len(v)
    w = np.zeros((16, L // 16), np.int16)
    w[np.arange(L) % 16, np.arange(L) // 16] = v.astype(np.int16)
    return np.tile(w, (8, 1))


def _bf16():
    try:
        return np.dtype("bfloat16")
    except TypeError:
        import ml_dtypes
        return ml_dtypes.bfloat16


def preprocess(inputs):
    src = np.asarray(inputs["src"]).astype(np.int64)
    dst = np.asarray(inputs["dst"]).astype(np.int64)
    graph_id = np.asarray(inputs["graph_id"]).astype(np.int64)
    feat = np.asarray(inputs["feat"], dtype=np.float32)
    bfnp = _bf16()

    per_core_edges = []
    KB = 0
    for c in range(NCORES):
        m = (dst >= c * SH) & (dst < (c + 1) * SH)
        es, ed = src[m], dst[m]
        o = np.argsort(ed, kind="stable")
        es, ed = es[o], ed[o]
        dl = ed - c * SH
        blk = dl >> 7
        counts = np.bincount(blk, minlength=NB)
        KB = max(KB, int(np.ceil(counts.max() / 128)))
        per_core_edges.append((es, ed, dl, blk, counts))
    EB = KB * 128

    # shared weight-derived arrays
    def Amat(al):  # [1,H,D] -> [H*D, H]
        al = np.asarray(al, np.float64)[0]
        hh, dd = al.shape
        A = np.zeros((hh * dd, hh), np.float64)
        for h in range(hh):
            A[h * dd:(h + 1) * dd, h] = al[h]
        return A

    W0 = np.asarray(inputs["W0"], np.float64)
    W1 = np.asarray(inputs["W1"], np.float64)
    W2 = np.asarray(inputs["W2"], np.float64)
    W3 = np.asarray(inputs["W3"], np.float64)
    resW3 = np.asarray(inputs["resW3"], np.float64)
    al3 = np.asarray(inputs["al3"], np.float64)[0, :, 0]
    ar3 = np.asarray(inputs["ar3"], np.float64)[0, :, 0]

    WA = {}
    for l, W in ((0, W0), (1, W1), (2, W2)):
        Aal = Amat(inputs[f"al{l}"])
        Aar = Amat(inputs[f"ar{l}"])
        WA[l] = np.concatenate([W @ Aal, W @ Aar], axis=1).astype(bfnp)
    W3c = np.concatenate(
        [W3, W3 * al3[None, :], W3 * ar3[None, :], resW3], axis=1
    ).astype(bfnp)  # [512, 24]
    b3row = np.zeros((1, 24), np.float32)
    b3row[0, 18:24] = np.asarray(inputs["b3"], np.float32)

    bias_bc = np.tile(
        np.concatenate([np.asarray(inputs[f"b{l}"], np.float32)
                        for l in range(3)])[None, :], (128, 1))  # [128, 3*512]
    lin_bc = np.zeros((128, H3 + 1), np.float32)
    lin_bc[:, 0:H3] = np.asarray(inputs["linW"], np.float32)[:, 0][None, :]
    lin_bc[:, H3] = float(np.asarray(inputs["linb"], np.float32)[0])

    shared = {
        "W0": W0.astype(bfnp),
        "W1": W1.astype(bfnp),
        "W2": W2.astype(bfnp),
        "W3c": W3c,
        "WA0": WA[0], "WA1": WA[1], "WA2": WA[2],
        "b3row": b3row.astype(bfnp),
        "bias_bc": bias_bc,
        "lin_bc": lin_bc,
        "identity": np.eye(128, dtype=bfnp),
        "ones1": np.ones((1, 128), bfnp),
    }

    in_maps = []
    eye64 = np.eye(G, dtype=np.float32)
    for c in range(NCORES):
        es, ed, dl, blk, counts = per_core_edges[c]
        offs = np.concatenate([[0], np.cumsum(counts)])
        idxX = np.zeros((128, NB * EB // 16), np.int16)
        idxE = np.zeros((128, NB * EB // 16), np.int16)
        Sarr = np.zeros((NB, 128, EB), np.float32)
        for b in range(NB):
            s_b = es[offs[b]:offs[b + 1]]
            d_b = ed[offs[b]:offs[b + 1]]
            dloc = dl[offs[b]:offs[b + 1]] - b * 128
            npad = EB - len(s_b)
            s_pad = np.concatenate([s_b, np.zeros(npad, np.int64)])
            dg_pad = np.concatenate([d_b, np.zeros(npad, np.int64)])
            idxX[:, b * EB // 16:(b + 1) * EB // 16] = _wrap_idx(s_pad)
            idxE[:, b * EB // 16:(b + 1) * EB // 16] = _wrap_idx(dg_pad)
            j = np.arange(len(dloc))
            S3 = Sarr[b].reshape(128, KB, 128)
            S3[j % 128, j // 128, dloc] = 1.0
        gid = graph_id[c * SH:(c + 1) * SH]
        Gh = eye64[gid].reshape(NB, 128, G)
        im = dict(shared)
        im["feat_sh"] = feat[c * SH:(c + 1) * SH].astype(bfnp)
        im["idxX"] = idxX
        im["idxE"] = idxE
        im["Sarr"] = Sarr.astype(bfnp)
        im["Gh"] = Gh
        in_maps.append(im)
    return in_maps, KB


@functools.lru_cache(maxsize=8)
def build_program(KB, phases=5, no_cc=False, ebufs=3, t2a_dve=True):
    EB = KB * 128
    nc = bacc.Bacc("TRN2", target_bir_lowering=False, debug=False)

    # ---- I/O ----
    feat_sh = nc.dram_tensor("feat_sh", [SH, F0], bf16, kind="ExternalInput")
    Wt = {
        0: nc.dram_tensor("W0", [F0, F], bf16, kind="ExternalInput"),
        1: nc.dram_tensor("W1", [F, F], bf16, kind="ExternalInput"),
        2: nc.dram_tensor("W2", [F, F], bf16, kind="ExternalInput"),
        3: nc.dram_tensor("W3c", [F, 24], bf16, kind="ExternalInput"),
    }
    WAt = {l: nc.dram_tensor(f"WA{l}", [F0 if l == 0 else F, 16], bf16,
                             kind="ExternalInput") for l in range(3)}
    b3row = nc.dram_tensor("b3row", [1, 24], bf16, kind="ExternalInput")
    bias_bc = nc.dram_tensor("bias_bc", [128, 3 * F], f32, kind="ExternalInput")
    lin_bc = nc.dram_tensor("lin_bc", [128, H3 + 1], f32, kind="ExternalInput")
    identity = nc.dram_tensor("identity", [128, 128], bf16, kind="ExternalInput")
    ones1 = nc.dram_tensor("ones1", [1, 128], bf16, kind="ExternalInput")
    idxX = nc.dram_tensor("idxX", [128, NB * EB // 16], i16, kind="ExternalInput")
    idxE = nc.dram_tensor("idxE", [128, NB * EB // 16], i16, kind="ExternalInput")
    Sarr = nc.dram_tensor("Sarr", [NB, 128, EB], bf16, kind="ExternalInput")
    Gh = nc.dram_tensor("Gh", [NB, 128, G], f32r, kind="ExternalInput")
    out = nc.dram_tensor("out", [G, 1], f32, kind="ExternalOutput")

    rg = [list(range(NCORES))]

    with tile.TileContext(nc) as tc:
        with (
            tc.tile_pool(name="const", bufs=1) as constp,
            tc.tile_pool(name="work", bufs=3) as work,
            tc.tile_pool(name="edge", bufs=ebufs) as edge,
            tc.tile_pool(name="psA", bufs=2, space="PSUM") as psA,
            tc.tile_pool(name="psB", bufs=2, space="PSUM") as psB,
            tc.tile_pool(name="psC", bufs=2, space="PSUM") as psC,
            tc.tile_pool(name="psP", bufs=1, space="PSUM") as psP,
            tc.tile_pool(name="dram", bufs=1, space="DRAM") as dram,
        ):
            # ---- resident constants ----
            ident_sb = constp.tile([128, 128], bf16)
            nc.sync.dma_start(ident_sb[:], identity[:])
            ones_sb = constp.tile([1, 128], bf16)
            nc.sync.dma_start(ones_sb[:], ones1[:])
            b3r_sb = constp.tile([1, 24], bf16)
            nc.sync.dma_start(b3r_sb[:], b3row[:])
            lin_sb = constp.tile([128, H3 + 1], f32)
            nc.sync.dma_start(lin_sb[:], lin_bc[:])
            bias_sb = constp.tile([128, 3 * F], f32)
            nc.sync.dma_start(bias_sb[:], bias_bc[:])
            res3_sb = constp.tile([128, NB * H3], f32)
            ixX_sb = constp.tile([128, NB * EB // 16], i16)
            nc.sync.dma_start(ixX_sb[:], idxX[:])
            ixE_sb = constp.tile([128, NB * EB // 16], i16)
            nc.sync.dma_start(ixE_sb[:], idxE[:])
            gh_sb = constp.tile([128, NB, G], f32r)
            nc.sync.dma_start(gh_sb[:], Gh[:].rearrange("n p g -> p n g"))
            w_sb = {}
            wa_sb = {}
            for l in range(4):
                K = F0 if l == 0 else F
                KBl = K // 128
                FW = F if l < 3 else 24
                w_sb[l] = constp.tile([128, KBl, FW], bf16, name=f"w{l}sb")
                nc.sync.dma_start(
                    w_sb[l][:], Wt[l][:].rearrange("(kb p) f -> p kb f", p=128))
                if l < 3:
                    wa_sb[l] = constp.tile([128, KBl, 16], bf16, name=f"wa{l}sb")
                    nc.sync.dma_start(
                        wa_sb[l][:],
                        WAt[l][:].rearrange("(kb p) f -> p kb f", p=128))

            # ---- resident node features (ping-pong) ----
            hA = constp.tile([128, NB, F], bf16)   # h1, then h3
            hB = constp.tile([128, NB, F], bf16)   # h2

            # ---- internal DRAM arrays ----
            ftag = {l: dram.tile([SH, FR], fp8, name=f"ftag{l}")
                    for l in range(3)}
            ftg = {l: dram.tile([N, FR], fp8, name=f"ftg{l}",
                                addr_space="Shared") for l in range(3)}
            ft3ag = dram.tile([SH, 64], f32, name="ft3ag")
            ft3g = dram.tile([N, 64], f32, name="ft3g", addr_space="Shared")
            ar_in = dram.tile([G, H3], f32, name="arin")
            ar_out = dram.tile([G, H3], f32, name="arout", addr_space="Shared")

            def hbuf(l):
                """SBUF buffer holding h_{l} (the INPUT of layer l)."""
                return (hA, hB, hA)[l - 1] if l >= 1 else None

            # ---- dense chunk: h(l) chunk j -> ftag[l] rows / ft3ag ----
            def dense_chunk(l, j):
                K = F0 if l == 0 else F
                KBl = K // 128
                if l == 0:
                    hsrc = work.tile([128, F0], bf16, tag="hload")
                    nc.sync.dma_start(hsrc[:], feat_sh[j * 128:(j + 1) * 128, :])
                else:
                    hsrc = hbuf(l)[:, j, :]
                pT = psA.tile([128, K], bf16, tag="A")
                for kb in range(KBl):
                    nc.tensor.transpose(
                        pT[:, kb * 128:(kb + 1) * 128],
                        hsrc[:, kb * 128:(kb + 1) * 128], ident_sb[:])
                hT = work.tile([128, K], bf16, tag="hT")
                nc.scalar.copy(hT[:], pT[:])
                FW = F if l < 3 else 24
                pft = psB.tile([128, FW], f32, tag="B")
                for kb in range(KBl):
                    nc.tensor.matmul(
                        pft[:], hT[:, kb * 128:(kb + 1) * 128],
                        w_sb[l][:, kb, :],
                        start=(kb == 0), stop=(kb == KBl - 1 and l < 3))
                if l == 3:
                    nc.tensor.matmul(pft[:], ones_sb[:], b3r_sb[:],
                                     start=False, stop=True)
                    ft3t = work.tile([128, 64], f32, tag="ft3sb")
                    nc.any.tensor_copy(ft3t[:, 0:18], pft[:, 0:18])
                    nc.vector.memset(ft3t[:, 18:64], 0.0)
                    nc.any.tensor_copy(
                        res3_sb[:, j * H3:(j + 1) * H3], pft[:, 18:24])
                    nc.sync.dma_start(ft3ag[j * 128:(j + 1) * 128, :], ft3t[:])
                else:
                    pel = psC.tile([128, 16], f32, tag="C")
                    for kb in range(KBl):
                        nc.tensor.matmul(
                            pel[:], hT[:, kb * 128:(kb + 1) * 128],
                            wa_sb[l][:, kb, :],
                            start=(kb == 0), stop=(kb == KBl - 1))
                    ftt = work.tile([128, FR], fp8, tag="ftsb")
                    nc.scalar.copy(ftt[:, 0:F], pft[:])
                    nc.scalar.copy(ftt[:, F:F + 32].bitcast(bf16), pel[:])
                    nc.vector.memset(ftt[:, F + 32:FR], 0.0)
                    nc.sync.dma_start(ftag[l][j * 128:(j + 1) * 128, :], ftt[:])

            def allgather(l):
                if no_cc:
                    return
                if l == 3:
                    nc.gpsimd.collective_compute(
                        "AllGather", ALU.bypass, replica_groups=rg,
                        ins=[ft3ag[:].opt()], outs=[ft3g[:].opt()])
                else:
                    nc.gpsimd.collective_compute(
                        "AllGather", ALU.bypass, replica_groups=rg,
                        ins=[ftag[l][:].opt()], outs=[ftg[l][:].opt()])

            # ---- edge block: aggregate layer l for dst block b -> h(l+1) ----
            def edge_block(l, b):
                S_t = edge.tile([128, KB, 128], bf16, tag="S")
                nc.sync.dma_start(
                    S_t[:], Sarr[b].rearrange("p (c d) -> p c d", d=128))
                X = edge.tile([128, KB, FR], fp8, tag="X")
                nc.gpsimd.dma_gather(
                    X[:], ftg[l][:], ixX_sb[:, b * EB // 16:(b + 1) * EB // 16],
                    num_idxs=EB, num_idxs_reg=EB, elem_size=FR,
                    single_packet=False)
                EL = edge.tile([128, KB, 256], fp8, tag="EL")
                nc.gpsimd.dma_gather(
                    EL[:], ftg[l][:, F:FR],
                    ixE_sb[:, b * EB // 16:(b + 1) * EB // 16],
                    num_idxs=EB, num_idxs_reg=EB, elem_size=256,
                    elem_step=FR, single_packet=False)
                elsrc = X[:, 0:KB, F:F + 32].bitcast(bf16)
                erdst = EL[:, 0:KB, 0:32].bitcast(bf16)
                et = edge.tile([128, KB, H], f32, tag="et")
                nc.vector.tensor_tensor(
                    et[:], elsrc[:, :, 0:H], erdst[:, :, H:2 * H], ALU.add)
                lt = edge.tile([128, KB * H], f32, tag="lt")
                nc.scalar.activation(
                    lt[:], et[:].rearrange("p c h -> p (c h)"), AF.Lrelu,
                    alpha=NEG_SLOPE)
                pt = edge.tile([128, KB * H], bf16, tag="pt")
                nc.scalar.activation(pt[:], lt[:], AF.Exp)
                Xv = X[:, :, 0:F].rearrange("p c (h d) -> p c h d", h=H)
                Xw = edge.tile([128, KB, F], bf16, tag="Xw")
                Xwv = Xw[:].rearrange("p c (h d) -> p c h d", h=H)
                pb = pt[:].rearrange("p (c h) -> p c h", h=H) \
                    .unsqueeze(3).broadcast_to([128, KB, H, D])
                nc.vector.tensor_tensor(Xwv, Xv, pb, ALU.mult)
                prst = psA.tile([128, F], f32, tag="A")
                ps = psC.tile([128, H], f32, tag="C")
                for c in range(KB):
                    nc.tensor.matmul(
                        prst[:], S_t[:, c, :], Xw[:, c, :],
                        start=(c == 0), stop=(c == KB - 1))
                for c in range(KB):
                    nc.tensor.matmul(
                        ps[:], S_t[:, c, :], pt[:, c * H:(c + 1) * H],
                        start=(c == 0), stop=(c == KB - 1))
                sse = edge.tile([128, H], f32, tag="sse")
                nc.vector.tensor_scalar_add(sse[:], ps[:], EPS)
                rs = edge.tile([128, H], f32, tag="rs")
                nc.vector.reciprocal(rs[:], sse[:])
                t1 = edge.tile([128, H, D], f32, tag="t1")
                nc.vector.tensor_tensor(
                    t1[:], prst[:].rearrange("p (h d) -> p h d", h=H),
                    rs[:].unsqueeze(2).broadcast_to([128, H, D]), ALU.mult)
                t1f = t1[:].rearrange("p h d -> p (h d)")
                t2 = edge.tile([128, F], f32, tag="t2")
                if l == 0:
                    nc.vector.tensor_tensor(
                        t2[:], t1f, bias_sb[:, l * F:(l + 1) * F], ALU.add)
                else:
                    t2a = edge.tile([128, F], f32, tag="t2a")
                    eng = nc.vector if t2a_dve else nc.gpsimd
                    eng.tensor_tensor(
                        t2a[:], t1f, hbuf(l)[:, b, :], ALU.add)
                    nc.vector.tensor_tensor(
                        t2[:], t2a[:], bias_sb[:, l * F:(l + 1) * F], ALU.add)
                # ELU -> h(l+1) resident in SBUF
                mm = edge.tile([128, F], f32, tag="mm")
                nc.vector.tensor_scalar_min(mm[:], t2[:], 0.0)
                ex = edge.tile([128, F], f32, tag="ex")
                nc.scalar.activation(ex[:], mm[:], AF.Exp)
                rl = edge.tile([128, F], f32, tag="rl")
                nc.scalar.activation(rl[:], t2[:], AF.Relu)
                nc.vector.scalar_tensor_tensor(
                    hbuf(l + 1)[:, b, :], ex[:], -1.0, rl[:], ALU.add, ALU.add)

            # ---- edge block layer 3 + pooling ----
            ppool = psP.tile([G, H3], f32, tag="P")

            def edge3_block(b):
                S_t = edge.tile([128, KB, 128], bf16, tag="S")
                nc.sync.dma_start(
                    S_t[:], Sarr[b].rearrange("p (c d) -> p c d", d=128))
                XS = edge.tile([128, KB, 64], f32, tag="X3")
                nc.gpsimd.dma_gather(
                    XS[:], ft3g[:], ixX_sb[:, b * EB // 16:(b + 1) * EB // 16],
                    num_idxs=EB, num_idxs_reg=EB, elem_size=64,
                    single_packet=False)
                EL = edge.tile([128, KB, 64], f32, tag="EL3")
                nc.gpsimd.dma_gather(
                    EL[:], ft3g[:], ixE_sb[:, b * EB // 16:(b + 1) * EB // 16],
                    num_idxs=EB, num_idxs_reg=EB, elem_size=64,
                    single_packet=False)
                et = edge.tile([128, KB, H3], f32, tag="et")
                nc.vector.tensor_tensor(
                    et[:], XS[:, 0:KB, 6:12], EL[:, 0:KB, 12:18], ALU.add)
                lt = edge.tile([128, KB * H3], f32, tag="lt")
                nc.vector.scalar_tensor_tensor(
                    lt[:], et[:].rearrange("p c h -> p (c h)"), NEG_SLOPE,
                    et[:].rearrange("p c h -> p (c h)"), ALU.mult, ALU.max)
                XP = edge.tile([128, KB, 2 * H3], bf16, tag="pt3")
                nc.scalar.activation(
                    XP[:, :, H3:2 * H3],
                    lt[:].rearrange("p (c h) -> p c h", h=H3), AF.Exp)
                nc.vector.tensor_tensor(
                    XP[:, :, 0:H3], XS[:, 0:KB, 0:H3], XP[:, :, H3:2 * H3],
                    ALU.mult)
                prst = psC.tile([128, 2 * H3], f32, tag="C")
                for c in range(KB):
                    nc.tensor.matmul(
                        prst[:], S_t[:, c, :], XP[:, c, :],
                        start=(c == 0), stop=(c == KB - 1))
                sse = edge.tile([128, H3], f32, tag="sse")
                nc.vector.tensor_scalar_add(sse[:], prst[:, H3:2 * H3], EPS)
                rs = edge.tile([128, H3], f32, tag="rs")
                nc.vector.reciprocal(rs[:], sse[:])
                t1 = edge.tile([128, H3], f32, tag="t1")
                nc.vector.tensor_tensor(t1[:], prst[:, 0:H3], rs[:], ALU.mult)
                h3 = edge.tile([128, H3], f32r, tag="hn3")
                nc.vector.tensor_tensor(
                    h3[:], t1[:], res3_sb[:, b * H3:(b + 1) * H3], ALU.add)
                nc.tensor.matmul(
                    ppool[:], gh_sb[:, b, :], h3[:],
                    start=(b == 0), stop=(b == NB - 1))

            def readout():
                pol = work.tile([G, H3], f32, tag="pol")
                nc.any.tensor_copy(pol[:], ppool[:])
                nc.sync.dma_start(ar_in[:], pol[:])
                nc.gpsimd.collective_compute(
                    "AllReduce", ALU.add, replica_groups=rg,
                    ins=[ar_in[:].opt()], outs=[ar_out[:].opt()])
                pol2 = work.tile([G, H3], f32, tag="pol2")
                nc.sync.dma_start(pol2[:], ar_out[:])
                pr = work.tile([G, H3], f32, tag="pr")
                nc.vector.tensor_tensor(
                    pr[:], pol2[:], lin_sb[0:G, 0:H3], ALU.mult)
                ro = work.tile([G, 1], f32, tag="ro")
                nc.vector.tensor_reduce(ro[:], pr[:], axis=AX.X, op=ALU.add)
                ro2 = work.tile([G, 1], f32, tag="ro2")
                nc.vector.tensor_tensor(
                    ro2[:], ro[:], lin_sb[0:G, H3:H3 + 1], ALU.add)
                nc.sync.dma_start(out[:], ro2[:])

            # ---- schedule ----
            def stage0():
                for j in range(NB):
                    dense_chunk(0, j)
                allgather(0)

            def stage(l):  # edge(l) blocks interleaved with dense(l+1),
                # dense shifted one block back so PE never stalls on the
                # current block's post-processing chain
                for b in range(NB):
                    edge_block(l, b)
                    if b >= 1:
                        dense_chunk(l + 1, b - 1)
                dense_chunk(l + 1, NB - 1)
                allgather(l + 1)

            def stage3():
                for b in range(NB):
                    edge3_block(b)
                readout()

            steps = [("dense0", stage0),
                     ("layer0", lambda: stage(0)),
                     ("layer1", lambda: stage(1)),
                     ("layer2", lambda: stage(2)),
                     ("layer3", stage3)]
            for nm, st in steps[:phases]:
                with nc.named_scope(nm):
                    st()

    nc.compile()
    return nc


def kernel(**inputs):
    in_maps, KB = preprocess(inputs)
    nc = build_program(KB, LAST.get("phases", 5), LAST.get("no_cc", False),
                       LAST.get("ebufs", 3), LAST.get("t2a_dve", True))
    br = run_bass_kernel_spmd(
        nc, in_maps, core_ids=list(range(NCORES)), trace=TRACE, **TRACE_KW)
    LAST["br"] = br
    return np.asarray(br.results[0]["out"], dtype=np.float32)
